# revision 11
# baseline (speedup 1.0000x reference)
"""Correlation-volume kernel for Trainium2 (8 NeuronCores, data-parallel over B).

corr[b, d, h, w] = sum_c L[b,h,w,c] * R[b,h,w-d,c], 0 <= d < 48, zero-padded w-d < 0.

Device strategy (per core = one batch):
  - Host shards each batch pre-transposed and L/R-interleaved to
    [C, H, 2, W] fp32, so the contraction dim C lands on SBUF partitions
    straight off the DMA -- no on-device transposes -- and each h-block
    needs ONE big SWDGE load (fp32 -> fp16 cast inline, contiguous
    per-partition descriptors).  Big DMAs + 4-deep load lookahead keep
    the 16 SDMA engines saturated (they are the roofline here).
  - Banded Gram tiles G[u, w] = sum_c R^T[c,u] * L^T[c,w] in u-chunks of
    32, four h rows packed onto the 128 PSUM partitions via col-tiling
    (tile_position=(0,32j) for row j). Valid band window w in
    [u0, u0+78] per chunk; 10 chunks split across two PSUM banks
    (395 + 333 fp32 cols).
  - Two DVE copies drain each 4-row group PSUM f32 -> SBUF f16 band;
    one f16 DMA per NH rows writes DRAM (7.5 MB total vs 9.8 MB dense).
  - Host extracts the 48 diagonals (corr[d,h,w] = G[w-d, w]) while
    unsharding and casts back to fp32: host-side glue, free for the
    device.
"""

import os
import sys

import numpy as np

for _p in (
    "/root/.axon_site",
    "/root/.axon_site/_ro/trn_rl_repo",
    "/root/.axon_site/_ro/pypackages",
    "/opt/trn_rl_repo",
    "/opt/pypackages",
):
    if os.path.isdir(_p) and _p not in sys.path:
        sys.path.append(_p)

import concourse.bacc as bacc
import concourse.mybir as mybir
import concourse.tile as tile
from concourse.bass_utils import run_bass_kernel_spmd

B, H, W, C, D = 8, 160, 320, 128, 48
NH = 20  # h rows per DMA block (multiple of 4)
NG = NH // 4  # 4-row groups per block
F32 = mybir.dt.float32
F16 = mybir.dt.float16

# u-chunks of 32: (u0, window width); window w in [u0, min(u0+32+47, W))
CHUNKS = [(32 * k, min(79, W - 32 * k)) for k in range(10)]
# chunks 0-4 drain from PSUM bank A, 5-9 from bank B (2 KiB bank limit)
NKA = 5
PSA = sum(wn for _, wn in CHUNKS[:NKA])  # 395 fp32 cols
PSB = sum(wn for _, wn in CHUNKS[NKA:])  # 333 fp32 cols
OFFB = 400  # bank-B cols start here in the f16 out slot (8B aligned)
OFFS = [0, 79, 158, 237, 316, OFFB, OFFB + 79, OFFB + 158, OFFB + 237, OFFB + 301]
SLOT = 736  # per-group col slot in the output block (f16)

_cache: dict = {}


def _build(h_run: int = H):
    nc = bacc.Bacc("TRN2", target_bir_lowering=False, debug=False, num_devices=B)
    # host-interleaved: LR[c, h, 0, w] = L^T, LR[c, h, 1, w] = R^T
    LR = nc.dram_tensor("LR", [C, H, 2, W], F32, kind="ExternalInput").ap()
    # [(j,u), g, col]: h = 4*g + j, chunk k at col = OFFS[k] + (w - u0_k)
    OUT = nc.dram_tensor("OUT", [128, H // 4, SLOT], F16, kind="ExternalOutput").ap()

    with tile.TileContext(nc) as tc:
        with (
            tc.tile_pool(name="loads", bufs=4) as lpool,
            tc.tile_pool(name="outbuf", bufs=3) as opool,
            tc.tile_pool(name="psum", bufs=4, space="PSUM") as ps_pool,
        ):
            for hb in range(0, h_run, NH):
                ng = NG
                nat = lpool.tile([C, NH, 2, W], F16, tag="nat")
                gout = opool.tile([128, NG, SLOT], F16, tag="gout")
                # SWDGE casts fp32 -> fp16 inline during the load
                nc.gpsimd.dma_start(
                    out=nat[:],
                    in_=LR[:, hb : hb + NH, :, :],
                )

                for g in range(ng):
                    pgA = ps_pool.tile([128, PSA], F32, tag="pgA")
                    pgB = ps_pool.tile([128, PSB], F32, tag="pgB")
                    for j in range(4):
                        hl = 4 * g + j
                        offA = offB = 0
                        for k, (u0, wn) in enumerate(CHUNKS):
                            if k < NKA:
                                dst = pgA[32 * j : 32 * j + 32, offA : offA + wn]
                                offA += wn
                            else:
                                dst = pgB[32 * j : 32 * j + 32, offB : offB + wn]
                                offB += wn
                            nc.tensor.matmul(
                                out=dst,
                                lhsT=nat[:, hl, 1, u0 : u0 + 32],
                                rhs=nat[:, hl, 0, u0 : u0 + wn],
                                start=True,
                                stop=True,
                                tile_position=(0, 32 * j),
                            )
                    # drain the group: PSUM f32 -> SBUF f16 band
                    nc.vector.tensor_copy(out=gout[:, g, 0:PSA], in_=pgA[:])
                    nc.vector.tensor_copy(
                        out=gout[:, g, OFFB : OFFB + PSB], in_=pgB[:]
                    )

                nc.sync.dma_start(
                    out=OUT[:, hb // 4 : hb // 4 + ng, :],
                    in_=gout[:],
                )

    nc.compile()
    return nc


def _get_nc(h_run: int = H):
    if h_run not in _cache:
        _cache[h_run] = _build(h_run)
    return _cache[h_run]


def _reconstruct(results) -> np.ndarray:
    """Assemble [B, D, H, W] from the per-core band blocks."""
    # X[b, (j,u), g, col] = corr[b, d, 4g+j, u0_k + ul + d] at
    # col = OFFS[k] + ul + d, partition = 32j + ul
    X = np.stack([r["OUT"] for r in results])  # [B, 128, H/4, SLOT] f16
    Xr = X.reshape(B, 4, 32, H // 4, SLOT).transpose(0, 3, 1, 2, 4)
    Xf = np.ascontiguousarray(Xr).reshape(B, H // 4, 4, 32 * SLOT)
    out = np.zeros((B, D, H, W), np.float32)
    ul = np.arange(32)
    for d in range(D):
        idx = ul * (SLOT + 1) + d
        for k, (u0, wn) in enumerate(CHUNKS):
            nu = min(32, W - u0 - d)
            if nu <= 0:
                continue
            v = Xf[:, :, :, OFFS[k] + idx[:nu]]  # [B, H/4, 4, nu] f16
            out[:, d, :, u0 + d : u0 + d + nu] = v.reshape(B, H, nu).astype(
                np.float32
            )
    return out


def _run(L_full, R_full, h_run: int = H, trace: bool = False):
    L_full = np.asarray(L_full, dtype=np.float32)
    R_full = np.asarray(R_full, dtype=np.float32)
    assert L_full.shape == (B, H, W, C), L_full.shape
    nc = _get_nc(h_run)
    # host-side shard: per-batch transpose [H, W, C] -> [C, H, W] and
    # interleave L/R rows -> [C, H, 2, W]
    LRI = np.ascontiguousarray(
        np.stack(
            [L_full.transpose(0, 3, 1, 2), R_full.transpose(0, 3, 1, 2)], axis=3
        )
    )
    in_maps = [{"LR": LRI[b]} for b in range(B)]
    res = run_bass_kernel_spmd(
        nc, in_maps, list(range(B)), trace=trace, trace_cores=[0] if trace else None
    )
    return _reconstruct(res.results), res


def kernel(L_corr, R_corr):
    out, _ = _run(L_corr, R_corr)
    return out
